# revision 1
# baseline (speedup 1.0000x reference)
"""GeneAwareContrastive loss — Trainium2 Bass kernel (8 NeuronCores, SPMD).

Math (equivalent to the nn.Module reference):
  fn    = l2-normalize rows of features            [B, D]
  sim   = (fn @ fn.T) / 0.5 = 2*G                  (bounded in [-2, 2])
  Since sim is bounded, logsumexp needs no max-shift:
     sumexp_neg_i = sum_j exp(sim_ij) - sum_{same-gene j (incl diag)} exp(sim_ij)
  within pair term (i<j orig order, same gene):
     softplus(lse_i - sim_ij) = Ln(exp(sim_ij) + sumexp_neg_i) - sim_ij
  cross term: relu(sim - margin) summed over different-gene pairs
            = 2*( relu(G - margin/2) summed over all - summed over same-gene )

Strategy:
  * stable-sort rows by gene on host -> same-gene pairs form a block-diagonal
    band; each 128-row tile's same-gene columns fit in a fixed W-wide window.
    Host precomputes the window offsets + same/triu masks (they fold in
    valid_gene, has_neg and the orig-index triangular condition).
  * shard rows across 8 cores (B/8 rows each); every core gets the full
    normalized-transposed feature matrix (8 MB) and computes its
    [B/8, B] slice of G on the PE in fp32r (inputs pre-rounded to fp22 on
    host, making the hardware truncation exact).
  * per 2048-col chunk: PE matmul -> PSUM; ACT does exp(2G) with fused
    row-sum (accum_out); DVE does relu(G-0.05) with fused row-sum.
  * per row-tile: recompute the W-wide window block, then 3 ACT/5 DVE ops
    produce the same-gene corrections and the within-pair sums.
  * each core returns [128, 4*T] per-partition partials; host reduces in
    float64 and assembles the 5 outputs. Label-only counts are computed
    exactly on host from the gene histogram.
"""

import os
import sys

import numpy as np

sys.path.insert(0, "/opt/trn_rl_repo")

TEMPERATURE = 0.5
W_WITHIN = 1.0
W_CROSS = 0.5
MARGIN = 0.1

N_CORES = 8
CH = 1024  # main column-chunk width (2 PSUM banks)

TRACE = False  # unused under axon (no NTFF hook); kept for compatibility
_LAST_RESULT = None
_LAST_RUN = None  # (fn, concat_in, concat_zeros, out_names, out_avals) for timing

_BUILD_CACHE = {}


def _relu_on_act(t, n, NCH):
    # which main chunks' relu+accum run on ACT (engine balance)
    return (t * NCH + n) % 32 == 5


def _round_fp22(x):
    """Round f32 array to nearest fp22 (e8m13) so PE fp32r truncation is exact."""
    i = x.astype(np.float32).view(np.uint32).astype(np.uint64)
    lsb = (i >> np.uint64(10)) & np.uint64(1)
    i = (i + np.uint64(0x1FF) + lsb) & np.uint64(0xFFFFFC00)
    return i.astype(np.uint32).view(np.float32)


def _build(B, D, RPC, W, ch):
    """Build + compile the per-core Bass/Tile program (identical on all cores)."""
    key = (B, D, RPC, W, ch)
    if key in _BUILD_CACHE:
        return _BUILD_CACHE[key]

    import concourse.bacc as bacc
    import concourse.tile as tile
    import concourse.mybir as mybir
    import concourse.hw_specs as _hw

    # Route Exp and Ln to the single combined table set so the ACT engine
    # loads one table once instead of thrashing exp<->ln sets every row tile.
    # Indices (act_func_set_id) are preserved; we only blank the contents of
    # the sets we don't want the greedy picker to choose.
    if not getattr(bacc, "_ant_act_tables_patched", False):
        _orig_tabs = _hw.get_activation_tables

        def _patched_tabs(arch):
            tabs = dict(_orig_tabs(arch))
            keep = "natural_log_exp_and_others"
            if keep in tabs:
                for k, fns in tabs.items():
                    if k != keep and (fns & tabs[keep]):
                        tabs[k] = set()
            return tabs

        bacc.get_activation_tables = _patched_tabs
        bacc._ant_act_tables_patched = True

    f32 = mybir.dt.float32
    f32r = mybir.dt.float32r
    Exp = mybir.ActivationFunctionType.Exp
    Ln = mybir.ActivationFunctionType.Ln
    Relu = mybir.ActivationFunctionType.Relu
    Alu = mybir.AluOpType
    X = mybir.AxisListType.X

    KC = D // 128  # contraction chunks
    T = RPC // 128  # row tiles per core
    NCH = B // ch  # main chunks per row tile
    SUB = ch // 512  # matmuls per chunk per k
    assert W <= 512 and ch % 512 == 0 and B % ch == 0 and RPC % 128 == 0 and D % 128 == 0

    nc = bacc.Bacc("TRN2", target_bir_lowering=False)

    rhs_d = nc.dram_tensor("rhs", [KC, 128, B], f32r, kind="ExternalInput")
    lhs_d = nc.dram_tensor("lhs", [KC, 128, RPC], f32r, kind="ExternalInput")
    win_d = nc.dram_tensor("win", [KC, 128, T * W], f32r, kind="ExternalInput")
    same_d = nc.dram_tensor("same", [T, 128, W], f32, kind="ExternalInput")
    triu_d = nc.dram_tensor("triu", [T, 128, W], f32, kind="ExternalInput")
    part_d = nc.dram_tensor("part", [128, 4 * T], f32, kind="ExternalOutput")

    with tile.TileContext(nc) as tc:
        with (
            tc.tile_pool(name="big", bufs=1) as big,
            tc.tile_pool(name="scr", bufs=2) as scr,
            tc.tile_pool(name="wscr", bufs=3) as wscr,
            tc.tile_pool(name="sums", bufs=3) as sums,
            tc.tile_pool(name="psum", bufs=3, space="PSUM") as psum,
            tc.tile_pool(name="psumw", bufs=2, space="PSUM") as psumw,
        ):
            rhs_sb = big.tile([128, KC, B], f32r)
            lhs_sb = big.tile([128, KC, RPC], f32r)
            win_sb = big.tile([128, KC, T * W], f32r)
            same_sb = big.tile([128, T, W], f32)
            triu_sb = big.tile([128, T, W], f32)
            part_sb = big.tile([128, 4 * T], f32)
            nbias = big.tile([128, 1], f32)  # -m/2 bias for ACT relu chunks
            nc.vector.memset(nbias, -MARGIN / 2)

            for k in range(KC):
                nc.sync.dma_start(out=lhs_sb[:, k, :], in_=lhs_d[k, :, :])
            for p0 in range(0, ch, 512):  # first chunk in small pieces
                for k in range(KC):
                    nc.sync.dma_start(
                        out=rhs_sb[:, k, p0 : p0 + 512], in_=rhs_d[k, :, p0 : p0 + 512]
                    )
            for p0 in range(ch, B, ch):
                for k in range(KC):
                    nc.sync.dma_start(
                        out=rhs_sb[:, k, p0 : p0 + ch], in_=rhs_d[k, :, p0 : p0 + ch]
                    )
            for k in range(KC):
                nc.sync.dma_start(out=win_sb[:, k, :], in_=win_d[k, :, :])
            for t in range(T):
                nc.sync.dma_start(out=same_sb[:, t, :], in_=same_d[t, :, :])
                nc.sync.dma_start(out=triu_sb[:, t, :], in_=triu_d[t, :, :])

            def emit_main(t):
                lhsT = [lhs_sb[:, k, t * 128 : (t + 1) * 128] for k in range(KC)]
                se = sums.tile([128, NCH], f32, tag="se")
                sr = sums.tile([128, NCH], f32, tag="sr")
                for n in range(NCH):
                    ps = psum.tile([128, ch], f32, tag="ps")
                    for sdx in range(SUB):
                        c0 = n * ch + sdx * 512
                        for k in range(KC):
                            nc.tensor.matmul(
                                ps[:, sdx * 512 : (sdx + 1) * 512],
                                lhsT[k],
                                rhs_sb[:, k, c0 : c0 + 512],
                                start=(k == 0),
                                stop=(k == KC - 1),
                            )
                    e_t = scr.tile([128, ch], f32, tag="e")
                    nc.scalar.activation(
                        out=e_t, in_=ps, func=Exp, scale=2.0,
                        accum_out=se[:, n : n + 1],
                    )
                    # accum semantics: accum_out = reduce(op1)( op0(in0, scalar1) )
                    # DVE chunks accumulate sum(max(G, m/2)); ACT chunks (engine
                    # balance) accumulate sum(relu(G - m/2)); host reconciles.
                    r_t = scr.tile([128, ch], f32, tag="r")
                    if _relu_on_act(t, n, NCH):
                        nc.scalar.activation(
                            out=r_t, in_=ps, func=Relu,
                            bias=nbias[:, :], scale=1.0,
                            accum_out=sr[:, n : n + 1],
                        )
                    else:
                        nc.vector.tensor_scalar(
                            out=r_t, in0=ps,
                            scalar1=MARGIN / 2, scalar2=None,
                            op0=Alu.max, op1=Alu.add,
                            accum_out=sr[:, n : n + 1],
                        )
                return lhsT, se, sr

            def emit_window(t, lhsT, se, sr):
                # same-gene corrections + within-pair sums over the W-window
                psw = psumw.tile([128, W], f32, tag="pw")
                for k in range(KC):
                    nc.tensor.matmul(
                        psw[:, :],
                        lhsT[k],
                        win_sb[:, k, t * W : (t + 1) * W],
                        start=(k == 0),
                        stop=(k == KC - 1),
                    )
                ew = wscr.tile([128, W], f32, tag="ew")
                nc.scalar.activation(out=ew, in_=psw[:, :], func=Exp, scale=2.0)
                es = sums.tile([128, 1], f32, tag="es")
                o1 = wscr.tile([128, W], f32, tag="o")
                nc.vector.scalar_tensor_tensor(
                    out=o1, in0=ew, scalar=0.0, in1=same_sb[:, t, :],
                    op0=Alu.add, op1=Alu.mult, accum_out=es,
                )
                sall = sums.tile([128, 1], f32, tag="sall")
                nc.vector.tensor_reduce(out=sall, in_=se, axis=X, op=Alu.add)
                sneg = sums.tile([128, 1], f32, tag="sneg")
                nc.vector.scalar_tensor_tensor(
                    out=sneg, in0=sall, scalar=1.0, in1=es,
                    op0=Alu.mult, op1=Alu.subtract,
                )
                lnw = wscr.tile([128, W], f32, tag="lnw")
                nc.scalar.activation(out=lnw, in_=ew, func=Ln, bias=sneg, scale=1.0)
                o2 = wscr.tile([128, W], f32, tag="o")
                nc.vector.scalar_tensor_tensor(
                    out=o2, in0=lnw, scalar=0.0, in1=triu_sb[:, t, :],
                    op0=Alu.add, op1=Alu.mult,
                    accum_out=part_sb[:, t : t + 1],
                )
                o3 = wscr.tile([128, W], f32, tag="o")
                nc.vector.scalar_tensor_tensor(
                    out=o3, in0=psw[:, :], scalar=2.0, in1=triu_sb[:, t, :],
                    op0=Alu.mult, op1=Alu.mult,
                    accum_out=part_sb[:, T + t : T + t + 1],
                )
                nc.vector.tensor_reduce(
                    out=part_sb[:, 2 * T + t : 2 * T + t + 1], in_=sr, axis=X,
                    op=Alu.add,
                )
                o4 = wscr.tile([128, W], f32, tag="o")
                nc.vector.scalar_tensor_tensor(
                    out=o4, in0=psw[:, :], scalar=MARGIN / 2, in1=same_sb[:, t, :],
                    op0=Alu.max, op1=Alu.mult,
                    accum_out=part_sb[:, 3 * T + t : 3 * T + t + 1],
                )

            # software-pipeline: emit window(t-1) between main(t) and main(t+1)
            # so the cross-engine window chain overlaps the next tile's bulk work.
            prev = None
            for t in range(T):
                cur = emit_main(t)
                if prev is not None:
                    emit_window(t - 1, *prev)
                prev = cur
            emit_window(T - 1, *prev)

            nc.sync.dma_start(out=part_d[:, :], in_=part_sb[:])

    nc.compile()
    _BUILD_CACHE[key] = nc
    return nc


_RUNNER_CACHE = {}


def _get_runner(key, nc):
    """Build (once) a jitted shard_map callable running the compiled Bass
    program SPMD on the 8 NeuronCores via the axon PJRT backend."""
    if key in _RUNNER_CACHE:
        return _RUNNER_CACHE[key]
    import jax
    from jax.experimental.shard_map import shard_map
    from jax.sharding import Mesh, PartitionSpec
    import concourse.mybir as mybir
    from concourse import bass2jax

    bass2jax.install_neuronx_cc_hook()

    partition_name = nc.partition_id_tensor.name if nc.partition_id_tensor else None
    in_names, out_names, out_avals, zero_outs = [], [], [], []
    for alloc in nc.m.functions[0].allocations:
        if not isinstance(alloc, mybir.MemoryLocationSet):
            continue
        name = alloc.memorylocations[0].name
        if alloc.kind == "ExternalInput":
            if name != partition_name:
                in_names.append(name)
        elif alloc.kind == "ExternalOutput":
            shape = tuple(alloc.tensor_shape)
            dtype = mybir.dt.np(alloc.dtype)
            out_names.append(name)
            out_avals.append(jax.core.ShapedArray(shape, dtype))
            zero_outs.append(np.zeros(shape, dtype))
    n_params = len(in_names)
    n_outs = len(out_avals)
    all_in_names = list(in_names) + list(out_names)
    if partition_name is not None:
        all_in_names.append(partition_name)

    def _body(*args):
        operands = list(args)
        if partition_name is not None:
            operands.append(bass2jax.partition_id_tensor())
        outs = bass2jax._bass_exec_p.bind(
            *operands,
            out_avals=tuple(out_avals),
            in_names=tuple(all_in_names),
            out_names=tuple(out_names),
            lowering_input_output_aliases=(),
            sim_require_finite=True,
            sim_require_nnan=True,
            nc=nc,
        )
        return tuple(outs)

    devices = jax.devices()[:N_CORES]
    mesh = Mesh(np.asarray(devices), ("core",))
    in_specs = (PartitionSpec("core"),) * (n_params + n_outs)
    out_specs = (PartitionSpec("core"),) * n_outs
    donate = tuple(range(n_params, n_params + n_outs))
    fn = jax.jit(
        shard_map(
            _body, mesh=mesh, in_specs=in_specs, out_specs=out_specs, check_rep=False
        ),
        donate_argnums=donate,
        keep_unused=True,
    )
    runner = (fn, in_names, out_names, out_avals, zero_outs)
    _RUNNER_CACHE[key] = runner
    return runner


def _run(nc, key, in_maps):
    """Execute on 8 cores; returns stacked 'part' outputs [N_CORES, 128, 4T]."""
    global _LAST_RUN
    fn, in_names, out_names, out_avals, zero_outs = _get_runner(key, nc)
    concat_in = [
        np.concatenate([in_maps[c][name] for c in range(N_CORES)], axis=0)
        for name in in_names
    ]
    concat_zeros = [
        np.zeros((N_CORES * z.shape[0], *z.shape[1:]), z.dtype) for z in zero_outs
    ]
    _LAST_RUN = (fn, concat_in, concat_zeros, out_names, out_avals)
    out_arrs = fn(*concat_in, *concat_zeros)
    i = out_names.index("part")
    a = np.asarray(out_arrs[i])
    return a.reshape(N_CORES, *out_avals[i].shape)


def _numpy_fallback(features, labs):
    """Direct numpy port of the reference (used only if structure assumptions fail)."""
    B = features.shape[0]
    fn = features / np.linalg.norm(features, axis=1, keepdims=True)
    sim = (fn @ fn.T) / TEMPERATURE
    same = labs[:, None] == labs[None, :]
    eye = np.eye(B, dtype=bool)
    same_off = same & ~eye
    neg = ~same
    has_neg = neg.any(axis=1)
    neg_sim = np.where(neg, sim, -np.inf)
    m = np.max(neg_sim, axis=1)
    m = np.where(np.isfinite(m), m, 0.0)
    lse = m + np.log(np.sum(np.where(neg, np.exp(neg_sim - m[:, None]), 0.0), axis=1))
    lse = np.where(has_neg, lse, 0.0)
    upper = np.triu(np.ones((B, B), dtype=bool), k=1)
    valid = (labs != -1)[:, None]
    pm = same_off & upper & valid & has_neg[:, None]
    z = lse[:, None] - sim
    within = np.where(pm, np.log1p(np.exp(-np.abs(z))) + np.maximum(z, 0), 0.0).sum()
    cross_cnt = int(neg.sum())
    cross_sum = np.where(neg, np.maximum(sim - MARGIN, 0.0), 0.0).sum()
    cross = cross_sum / cross_cnt if cross_cnt > 0 else 0.0
    total = W_WITHIN * within + W_CROSS * cross
    nw = float(same_off.sum())
    idt = np.int64 if labs.dtype == np.int64 else np.int32
    return (
        np.float32(total), np.float32(within), np.float32(cross),
        np.float32(nw), idt(cross_cnt),
    )


def kernel(**inputs):
    global _LAST_RESULT
    features = np.asarray(inputs["features"]).astype(np.float32, copy=False)
    labs_in = np.asarray(inputs["gene_labels"])
    labs = labs_in.astype(np.int64)
    B, D = features.shape

    ok = (
        B % (N_CORES * 128) == 0
        and D % 128 == 0
        and B % CH == 0
        and labs.shape == (B,)
    )
    if not ok:
        return _numpy_fallback(features, labs_in)

    RPC = B // N_CORES
    T = RPC // 128
    KC = D // 128
    NT = B // 128

    # ---- host prep: sort by gene, normalize, fp22 pre-round, masks ----
    perm = np.argsort(labs, kind="stable")
    fs = features[perm]
    ls = labs[perm]
    norm = np.sqrt((fs * fs).sum(axis=1, dtype=np.float32))
    with np.errstate(divide="ignore", invalid="ignore"):
        fn = fs / norm[:, None]
    fn22 = _round_fp22(fn)

    nlab = int(ls.max()) + 1 if ls.size else 1
    shifted = ls - ls.min() if ls.min() < 0 else ls
    nlab = int(shifted.max()) + 1
    counts = np.bincount(shifted, minlength=nlab)
    starts = np.concatenate([[0], np.cumsum(counts)])
    blk_start = starts[shifted]
    blk_end = blk_start + counts[shifted]
    cnt_row = counts[shifted]  # same-gene count (incl self) per sorted row
    has_neg = (B - cnt_row) > 0
    valid = ls != -1

    maxw = max(int(blk_end[t * 128 + 127] - blk_start[t * 128]) for t in range(NT))
    W = max(128, 64 * ((maxw + 63) // 64))
    if W > 512:
        return _numpy_fallback(features, labs_in)

    w0 = np.minimum(blk_start[:: 128], B - W).astype(np.int64)

    rows_all = np.arange(B)
    same_m = np.empty((NT, 128, W), np.float32)
    triu_m = np.empty((NT, 128, W), np.float32)
    for t in range(NT):
        rows = rows_all[t * 128 : (t + 1) * 128]
        cols = w0[t] + np.arange(W)
        sm = ls[cols][None, :] == ls[rows][:, None]
        same_m[t] = sm
        triu_m[t] = (
            sm
            & (cols[None, :] > rows[:, None])
            & valid[rows][:, None]
            & has_neg[rows][:, None]
        )

    fnT = np.ascontiguousarray(fn22.T).reshape(KC, 128, B)
    cidx = (w0[:, None] + np.arange(W)[None, :]).reshape(-1)
    win_all = np.ascontiguousarray(fnT[:, :, cidx])  # [KC, 128, NT*W]

    nc = _build(B, D, RPC, W, CH)

    in_maps = []
    for c in range(N_CORES):
        in_maps.append(
            {
                "rhs": fnT,
                "lhs": np.ascontiguousarray(fnT[:, :, c * RPC : (c + 1) * RPC]),
                "win": np.ascontiguousarray(
                    win_all[:, :, c * T * W : (c + 1) * T * W]
                ),
                "same": np.ascontiguousarray(same_m[c * T : (c + 1) * T]),
                "triu": np.ascontiguousarray(triu_m[c * T : (c + 1) * T]),
            }
        )

    parts = _run(nc, (B, D, RPC, W, CH), in_maps)  # [N_CORES, 128, 4T]

    # ---- host combine (float64) ----
    wa = np.zeros(B); wb = np.zeros(B); ra = np.zeros(B); rs_raw = np.zeros(B)
    for c in range(N_CORES):
        p = parts[c].astype(np.float64)  # [128, 4T]
        for t in range(T):
            sl = slice(c * RPC + t * 128, c * RPC + (t + 1) * 128)
            wa[sl] = p[:, t]
            wb[sl] = p[:, T + t]
            ra[sl] = p[:, 2 * T + t]
            rs_raw[sl] = p[:, 3 * T + t]

    # DVE relu chunks accumulate sum(max(G, m/2)) (= sum(relu(G-m/2)) + m/2*ch),
    # ACT relu chunks accumulate sum(relu(G - m/2)); rs is in max-form over
    # cnt_row same-gene columns. Reconcile the m/2 offsets per row.
    NCH = B // CH
    act_cnt = np.array(
        [sum(1 for n in range(NCH) if _relu_on_act(t, n, NCH)) for t in range(T)]
    )
    tile_of_row = (np.arange(B) % RPC) // 128
    dve_cols = B - act_cnt[tile_of_row] * CH
    within = (wa - wb).sum()
    sq = int((counts.astype(np.int64) ** 2).sum())
    cross_cnt = B * B - sq
    cross_sum = 2.0 * (ra - rs_raw - (MARGIN / 2) * (dve_cols - cnt_row)).sum()
    cross = cross_sum / cross_cnt if cross_cnt > 0 else 0.0
    total = W_WITHIN * within + W_CROSS * cross
    nw = float(sq - B)
    idt = np.int64 if labs_in.dtype == np.int64 else np.int32
    return (
        np.float32(total), np.float32(within), np.float32(cross),
        np.float32(nw), idt(cross_cnt),
    )



# revision 2
# speedup vs baseline: 595.3752x; 595.3752x over previous
"""GeneAwareContrastive loss — Trainium2 Bass kernel (8 NeuronCores, SPMD).

Math (equivalent to the nn.Module reference):
  fn    = l2-normalize rows of features            [B, D]
  sim   = (fn @ fn.T) / 0.5 = 2*G                  (bounded in [-2, 2])
  Since sim is bounded, logsumexp needs no max-shift:
     sumexp_neg_i = sum_j exp(sim_ij) - sum_{same-gene j (incl diag)} exp(sim_ij)
  within pair term (i<j orig order, same gene):
     softplus(lse_i - sim_ij) = Ln(exp(sim_ij) + sumexp_neg_i) - sim_ij
  cross term: relu(sim - margin) summed over different-gene pairs
            = 2*( max(G, m/2) summed over all - summed over same-gene
                  - m/2 * #neg )

Strategy:
  * stable-sort rows by gene on host -> same-gene pairs form a block-diagonal
    band; each 128-row tile's same-gene columns fit in a fixed W-wide window
    around the diagonal.
  * shard rows across 8 cores (B/8 rows each); every core gets the full
    normalized-transposed feature matrix in bf16, ROLLED by (core*RPC - pad)
    columns so tile t's window is the static slice [t*128, t*128+W) on every
    core -- no gathered window tensor, no extra DMA.
  * per 2048-col chunk (4 PSUM banks, pool bufs=2 -> 8 banks): k-outer
    matmuls (weight-stationary pairs) -> PSUM; ACT exp(2G) with fused row-sum
    (accum_out); DVE max(G, m/2) with fused row-sum.
  * per row-tile: W-wide window block recomputed on PE, then the same-gene
    corrections: ACT does exp/Ln, DVE does the PSUM-reading ops, GPSIMD does
    the SBUF-only mask multiplies (es, sneg, o2) to offload DVE.
  * a short burst of dummy matmuls at kernel start keeps the PE busy while
    the input DMAs land, so the HAM clock-gate reaches 8/8 before real work.
  * each core returns [128, 4*T] per-partition partials; host reduces in
    float64 and assembles the 5 outputs. Label-only counts are computed
    exactly on host from the gene histogram.
"""

import os
import sys

import numpy as np

sys.path.insert(0, "/opt/trn_rl_repo")

TEMPERATURE = 0.5
W_WITHIN = 1.0
W_CROSS = 0.5
MARGIN = 0.1

N_CORES = 8
CH = 2048  # main column-chunk width (4 PSUM banks)

_LAST_RESULT = None
_LAST_RUN = None  # (fn, concat_in, concat_zeros, out_names, out_avals) for timing

_BUILD_CACHE = {}


def _build(B, D, RPC, W, ch):
    """Build + compile the per-core Bass/Tile program (identical on all cores)."""
    key = (B, D, RPC, W, ch)
    if key in _BUILD_CACHE:
        return _BUILD_CACHE[key]

    import concourse.bacc as bacc
    import concourse.tile as tile
    import concourse.mybir as mybir
    import concourse.hw_specs as _hw

    # Route Exp and Ln to the single combined table set so the ACT engine
    # loads one table once instead of thrashing exp<->ln sets every row tile.
    if not getattr(bacc, "_ant_act_tables_patched", False):
        _orig_tabs = _hw.get_activation_tables

        def _patched_tabs(arch):
            tabs = dict(_orig_tabs(arch))
            keep = "natural_log_exp_and_others"
            if keep in tabs:
                for k, fns in tabs.items():
                    if k != keep and (fns & tabs[keep]):
                        tabs[k] = set()
            return tabs

        bacc.get_activation_tables = _patched_tabs
        bacc._ant_act_tables_patched = True

    f32 = mybir.dt.float32
    bf16 = mybir.dt.bfloat16
    Exp = mybir.ActivationFunctionType.Exp
    Ln = mybir.ActivationFunctionType.Ln
    Alu = mybir.AluOpType
    X = mybir.AxisListType.X

    KC = D // 128  # contraction chunks
    T = RPC // 128  # row tiles per core
    NCH = B // ch  # main chunks per row tile
    SUB = ch // 512  # matmuls per chunk per k
    assert W <= 512 and ch % 512 == 0 and B % ch == 0 and RPC % 128 == 0 and D % 128 == 0
    assert (T - 1) * 128 + W <= ch, "windows must lie inside rhs chunk 0"

    nc = bacc.Bacc("TRN2", target_bir_lowering=False)

    rhs_d = nc.dram_tensor("rhs", [KC, 128, B], bf16, kind="ExternalInput")
    lhs_d = nc.dram_tensor("lhs", [KC, 128, RPC], bf16, kind="ExternalInput")
    same_d = nc.dram_tensor("same", [T, 128, W], bf16, kind="ExternalInput")
    triu_d = nc.dram_tensor("triu", [T, 128, W], bf16, kind="ExternalInput")
    part_d = nc.dram_tensor("part", [128, 4 * T], f32, kind="ExternalOutput")

    with tile.TileContext(nc) as tc:
        with (
            tc.tile_pool(name="big", bufs=1) as big,
            tc.tile_pool(name="escr", bufs=2) as escr,
            tc.tile_pool(name="rscr", bufs=2) as rscr,
            tc.tile_pool(name="wscr", bufs=2) as wscr,
            tc.tile_pool(name="sums", bufs=3) as sums,
            tc.tile_pool(name="psum", bufs=2, space="PSUM") as psum,
        ):
            rhs_sb = big.tile([128, KC, B], bf16)
            lhs_sb = big.tile([128, KC, RPC], bf16)
            same_sb = big.tile([128, T, W], bf16)
            triu_sb = big.tile([128, T, W], bf16)
            part_sb = big.tile([128, 4 * T], f32)
            dve_scr = big.tile([128, W], f32)
            gps_scr = big.tile([128, W], f32)

            # PE warmup: dummy matmuls keep the PE busy from t=0 while the
            # input DMAs stream in, so HAM un-throttles (4/8 -> 8/8) before
            # the real matmuls start.
            warm_w = big.tile([128, 128], bf16)
            warm_x = big.tile([128, 512], bf16)
            nc.vector.memset(warm_w, 0.0)
            nc.vector.memset(warm_x, 0.0)
            wps = psum.tile([128, ch], f32, tag="ps")
            for w in range(10):
                nc.tensor.matmul(
                    wps[:, (w % SUB) * 512 : (w % SUB) * 512 + 512],
                    warm_w[:, :],
                    warm_x[:, :],
                    start=True,
                    stop=True,
                )

            # input DMAs, in consumption order
            for k in range(KC):
                nc.sync.dma_start(out=lhs_sb[:, k, :], in_=lhs_d[k, :, :])
            for p0 in range(0, ch, 512):  # first chunk in small pieces
                for k in range(KC):
                    nc.sync.dma_start(
                        out=rhs_sb[:, k, p0 : p0 + 512], in_=rhs_d[k, :, p0 : p0 + 512]
                    )
            for p0 in range(ch, 2 * ch, ch):
                for k in range(KC):
                    nc.sync.dma_start(
                        out=rhs_sb[:, k, p0 : p0 + ch], in_=rhs_d[k, :, p0 : p0 + ch]
                    )
            for t in range(T):
                nc.sync.dma_start(out=same_sb[:, t, :], in_=same_d[t, :, :])
                nc.sync.dma_start(out=triu_sb[:, t, :], in_=triu_d[t, :, :])
            for p0 in range(2 * ch, B, ch):
                for k in range(KC):
                    nc.sync.dma_start(
                        out=rhs_sb[:, k, p0 : p0 + ch], in_=rhs_d[k, :, p0 : p0 + ch]
                    )

            def emit_main(t):
                lhsT = [lhs_sb[:, k, t * 128 : (t + 1) * 128] for k in range(KC)]
                se = sums.tile([128, NCH], f32, tag="se")
                sr = sums.tile([128, NCH], f32, tag="sr")
                for n in range(NCH):
                    ps = psum.tile([128, ch], f32, tag="ps")
                    # k-outer: consecutive matmuls share the stationary operand
                    for k in range(KC):
                        for sdx in range(SUB):
                            c0 = n * ch + sdx * 512
                            nc.tensor.matmul(
                                ps[:, sdx * 512 : (sdx + 1) * 512],
                                lhsT[k],
                                rhs_sb[:, k, c0 : c0 + 512],
                                start=(k == 0),
                                stop=(k == KC - 1),
                            )
                    e_t = escr.tile([128, ch], bf16, tag="e")
                    nc.scalar.activation(
                        out=e_t, in_=ps, func=Exp, scale=2.0,
                        accum_out=se[:, n : n + 1],
                    )
                    # accum semantics: accum_out = reduce(op1)( op0(in0, scalar1) )
                    # accumulates sum(max(G, m/2)); host reconciles the m/2 bias.
                    r_t = rscr.tile([128, ch], bf16, tag="r")
                    nc.vector.tensor_scalar(
                        out=r_t, in0=ps,
                        scalar1=MARGIN / 2, scalar2=None,
                        op0=Alu.max, op1=Alu.add,
                        accum_out=sr[:, n : n + 1],
                    )
                return lhsT, se, sr

            def emit_window(t, lhsT, se, sr):
                # same-gene corrections + within-pair sums over the W-window
                psw = psum.tile([128, ch], f32, tag="ps")
                for k in range(KC):
                    nc.tensor.matmul(
                        psw[:, :W],
                        lhsT[k],
                        rhs_sb[:, k, t * 128 : t * 128 + W],
                        start=(k == 0),
                        stop=(k == KC - 1),
                    )
                ew = wscr.tile([128, W], f32, tag="ew")
                nc.scalar.activation(out=ew, in_=psw[:, :W], func=Exp, scale=2.0)
                es = sums.tile([128, 1], f32, tag="es")
                nc.gpsimd.scalar_tensor_tensor(
                    out=gps_scr, in0=ew, scalar=0.0, in1=same_sb[:, t, :],
                    op0=Alu.add, op1=Alu.mult, accum_out=es,
                )
                sall = sums.tile([128, 1], f32, tag="sall")
                nc.vector.tensor_reduce(out=sall, in_=se, axis=X, op=Alu.add)
                sneg = sums.tile([128, 1], f32, tag="sneg")
                nc.gpsimd.scalar_tensor_tensor(
                    out=sneg, in0=sall, scalar=1.0, in1=es,
                    op0=Alu.mult, op1=Alu.subtract,
                )
                lnw = wscr.tile([128, W], f32, tag="lnw")
                nc.scalar.activation(out=lnw, in_=ew, func=Ln, bias=sneg, scale=1.0)
                nc.gpsimd.scalar_tensor_tensor(
                    out=gps_scr, in0=lnw, scalar=0.0, in1=triu_sb[:, t, :],
                    op0=Alu.add, op1=Alu.mult,
                    accum_out=part_sb[:, t : t + 1],
                )
                nc.vector.scalar_tensor_tensor(
                    out=dve_scr, in0=psw[:, :W], scalar=2.0, in1=triu_sb[:, t, :],
                    op0=Alu.mult, op1=Alu.mult,
                    accum_out=part_sb[:, T + t : T + t + 1],
                )
                nc.vector.tensor_reduce(
                    out=part_sb[:, 2 * T + t : 2 * T + t + 1], in_=sr, axis=X,
                    op=Alu.add,
                )
                nc.vector.scalar_tensor_tensor(
                    out=dve_scr, in0=psw[:, :W], scalar=MARGIN / 2,
                    in1=same_sb[:, t, :],
                    op0=Alu.max, op1=Alu.mult,
                    accum_out=part_sb[:, 3 * T + t : 3 * T + t + 1],
                )

            # software-pipeline: emit window(t-1) between main(t) and main(t+1)
            # so the cross-engine window chain overlaps the next tile's bulk work.
            prev = None
            for t in range(T):
                cur = emit_main(t)
                if prev is not None:
                    emit_window(t - 1, *prev)
                prev = cur
            emit_window(T - 1, *prev)

            nc.sync.dma_start(out=part_d[:, :], in_=part_sb[:])

    nc.compile()
    _BUILD_CACHE[key] = nc
    return nc


_RUNNER_CACHE = {}


def _get_runner(key, nc):
    """Build (once) a jitted shard_map callable running the compiled Bass
    program SPMD on the 8 NeuronCores via the axon PJRT backend."""
    if key in _RUNNER_CACHE:
        return _RUNNER_CACHE[key]
    import jax
    from jax.experimental.shard_map import shard_map
    from jax.sharding import Mesh, PartitionSpec
    import concourse.mybir as mybir
    from concourse import bass2jax

    bass2jax.install_neuronx_cc_hook()

    partition_name = nc.partition_id_tensor.name if nc.partition_id_tensor else None
    in_names, out_names, out_avals, zero_outs = [], [], [], []
    for alloc in nc.m.functions[0].allocations:
        if not isinstance(alloc, mybir.MemoryLocationSet):
            continue
        name = alloc.memorylocations[0].name
        if alloc.kind == "ExternalInput":
            if name != partition_name:
                in_names.append(name)
        elif alloc.kind == "ExternalOutput":
            shape = tuple(alloc.tensor_shape)
            dtype = mybir.dt.np(alloc.dtype)
            out_names.append(name)
            out_avals.append(jax.core.ShapedArray(shape, dtype))
            zero_outs.append(np.zeros(shape, dtype))
    n_params = len(in_names)
    n_outs = len(out_avals)
    all_in_names = list(in_names) + list(out_names)
    if partition_name is not None:
        all_in_names.append(partition_name)

    def _body(*args):
        operands = list(args)
        if partition_name is not None:
            operands.append(bass2jax.partition_id_tensor())
        outs = bass2jax._bass_exec_p.bind(
            *operands,
            out_avals=tuple(out_avals),
            in_names=tuple(all_in_names),
            out_names=tuple(out_names),
            lowering_input_output_aliases=(),
            sim_require_finite=True,
            sim_require_nnan=True,
            nc=nc,
        )
        return tuple(outs)

    devices = jax.devices()[:N_CORES]
    mesh = Mesh(np.asarray(devices), ("core",))
    in_specs = (PartitionSpec("core"),) * (n_params + n_outs)
    out_specs = (PartitionSpec("core"),) * n_outs
    donate = tuple(range(n_params, n_params + n_outs))
    fn = jax.jit(
        shard_map(
            _body, mesh=mesh, in_specs=in_specs, out_specs=out_specs, check_rep=False
        ),
        donate_argnums=donate,
        keep_unused=True,
    )
    runner = (fn, in_names, out_names, out_avals, zero_outs)
    _RUNNER_CACHE[key] = runner
    return runner


def _run(nc, key, in_maps):
    """Execute on 8 cores; returns stacked 'part' outputs [N_CORES, 128, 4T]."""
    global _LAST_RUN
    fn, in_names, out_names, out_avals, zero_outs = _get_runner(key, nc)
    concat_in = [
        np.concatenate([in_maps[c][name] for c in range(N_CORES)], axis=0)
        for name in in_names
    ]
    concat_zeros = [
        np.zeros((N_CORES * z.shape[0], *z.shape[1:]), z.dtype) for z in zero_outs
    ]
    _LAST_RUN = (fn, concat_in, concat_zeros, out_names, out_avals)
    out_arrs = fn(*concat_in, *concat_zeros)
    i = out_names.index("part")
    a = np.asarray(out_arrs[i])
    return a.reshape(N_CORES, *out_avals[i].shape)


def _numpy_fallback(features, labs):
    """Direct numpy port of the reference (used only if structure assumptions fail)."""
    B = features.shape[0]
    fn = features / np.linalg.norm(features, axis=1, keepdims=True)
    sim = (fn @ fn.T) / TEMPERATURE
    same = labs[:, None] == labs[None, :]
    eye = np.eye(B, dtype=bool)
    same_off = same & ~eye
    neg = ~same
    has_neg = neg.any(axis=1)
    neg_sim = np.where(neg, sim, -np.inf)
    m = np.max(neg_sim, axis=1)
    m = np.where(np.isfinite(m), m, 0.0)
    lse = m + np.log(np.sum(np.where(neg, np.exp(neg_sim - m[:, None]), 0.0), axis=1))
    lse = np.where(has_neg, lse, 0.0)
    upper = np.triu(np.ones((B, B), dtype=bool), k=1)
    valid = (labs != -1)[:, None]
    pm = same_off & upper & valid & has_neg[:, None]
    z = lse[:, None] - sim
    within = np.where(pm, np.log1p(np.exp(-np.abs(z))) + np.maximum(z, 0), 0.0).sum()
    cross_cnt = int(neg.sum())
    cross_sum = np.where(neg, np.maximum(sim - MARGIN, 0.0), 0.0).sum()
    cross = cross_sum / cross_cnt if cross_cnt > 0 else 0.0
    total = W_WITHIN * within + W_CROSS * cross
    nw = float(same_off.sum())
    idt = np.int64 if labs.dtype == np.int64 else np.int32
    return (
        np.float32(total), np.float32(within), np.float32(cross),
        np.float32(nw), idt(cross_cnt),
    )


def kernel(**inputs):
    global _LAST_RESULT
    import concourse.mybir as mybir

    features = np.asarray(inputs["features"]).astype(np.float32, copy=False)
    labs_in = np.asarray(inputs["gene_labels"])
    labs = labs_in.astype(np.int64)
    B, D = features.shape

    ok = (
        B % (N_CORES * 128) == 0
        and D % 128 == 0
        and B % CH == 0
        and labs.shape == (B,)
    )
    if not ok:
        return _numpy_fallback(features, labs_in)

    RPC = B // N_CORES
    T = RPC // 128
    KC = D // 128
    NT = B // 128

    # ---- host prep: sort by gene, normalize, bf16 round, masks ----
    perm = np.argsort(labs, kind="stable")
    fs = features[perm]
    ls = labs[perm]
    norm = np.sqrt((fs * fs).sum(axis=1, dtype=np.float32))
    with np.errstate(divide="ignore", invalid="ignore"):
        fn = fs / norm[:, None]

    shifted = ls - ls.min() if ls.min() < 0 else ls
    nlab = int(shifted.max()) + 1
    counts = np.bincount(shifted, minlength=nlab)
    starts = np.concatenate([[0], np.cumsum(counts)])
    blk_start = starts[shifted]
    blk_end = blk_start + counts[shifted]
    cnt_row = counts[shifted]  # same-gene count (incl self) per sorted row
    has_neg = (B - cnt_row) > 0
    valid = ls != -1

    rows_all = np.arange(B)
    back = max(int(g * 128 - blk_start[g * 128]) for g in range(NT))
    fwd = max(int(blk_end[g * 128 + 127] - (g + 1) * 128) for g in range(NT))
    pad = 64 * ((max(back, 0) + 63) // 64)
    W = 128 + pad + 64 * ((max(fwd, 0) + 63) // 64)
    if W > 512 or (T - 1) * 128 + W > CH:
        return _numpy_fallback(features, labs_in)

    bf16 = mybir.dt.np(mybir.dt.bfloat16)
    same_m = np.empty((NT, 128, W), bf16)
    triu_m = np.empty((NT, 128, W), bf16)
    for g in range(NT):
        rows = rows_all[g * 128 : (g + 1) * 128]
        cols = (g * 128 - pad + np.arange(W)) % B
        sm = ls[cols][None, :] == ls[rows][:, None]
        same_m[g] = sm
        triu_m[g] = (
            sm
            & (cols[None, :] > rows[:, None])
            & valid[rows][:, None]
            & has_neg[rows][:, None]
        )

    fnT = np.ascontiguousarray(fn.T.astype(bf16)).reshape(KC, 128, B)

    nc = _build(B, D, RPC, W, CH)

    in_maps = []
    for c in range(N_CORES):
        roll = c * RPC - pad
        rhs_c = np.ascontiguousarray(np.roll(fnT, -roll, axis=2))
        in_maps.append(
            {
                "rhs": rhs_c,
                "lhs": np.ascontiguousarray(rhs_c[:, :, pad : pad + RPC]),
                "same": np.ascontiguousarray(same_m[c * T : (c + 1) * T]),
                "triu": np.ascontiguousarray(triu_m[c * T : (c + 1) * T]),
            }
        )

    parts = _run(nc, (B, D, RPC, W, CH), in_maps)  # [N_CORES, 128, 4T]

    # ---- host combine (float64) ----
    wa = np.zeros(B); wb = np.zeros(B); ra = np.zeros(B); rs_raw = np.zeros(B)
    for c in range(N_CORES):
        p = parts[c].astype(np.float64)  # [128, 4T]
        for t in range(T):
            sl = slice(c * RPC + t * 128, c * RPC + (t + 1) * 128)
            wa[sl] = p[:, t]
            wb[sl] = p[:, T + t]
            ra[sl] = p[:, 2 * T + t]
            rs_raw[sl] = p[:, 3 * T + t]

    # DVE relu accumulates sum(max(G, m/2)) over all B columns; the window
    # pass gives sum over the cnt_row same-gene columns. Per row:
    #   sum_neg relu(G - m/2) = (ra - rs_raw) - m/2 * (B - cnt_row)
    within = (wa - wb).sum()
    sq = int((counts.astype(np.int64) ** 2).sum())
    cross_cnt = B * B - sq
    cross_sum = 2.0 * (ra - rs_raw - (MARGIN / 2) * (B - cnt_row)).sum()
    cross = cross_sum / cross_cnt if cross_cnt > 0 else 0.0
    total = W_WITHIN * within + W_CROSS * cross
    nw = float(sq - B)
    idt = np.int64 if labs_in.dtype == np.int64 else np.int32
    return (
        np.float32(total), np.float32(within), np.float32(cross),
        np.float32(nw), idt(cross_cnt),
    )
